# revision 22
# baseline (speedup 1.0000x reference)
"""Trainium2 Bass kernel for a 4-step differentiable recurrent net forward pass.

Reference computation (B=8192, NI=512, NH=2048, NO=512, 4 steps):
    activs = 0; outputs = 0
    repeat 4x:  pre = hr * (x @ Wih.T + activs @ Whh.T + outputs @ Woh.T) + hb
                activs = per_neuron_act(pre)        # tanh/sigmoid/relu by i%3
    out = sigmoid(or * (x @ Wio.T + outputs @ Woo.T + activs @ Who.T) + ob)

`outputs` is never written inside the loop, so the Woh/Woo terms vanish and
the x-projection P = hr*(x@Wih.T)+hb is loop-invariant (computed once).

Strategy: data-parallel on batch across 8 cores (1024 rows each). On-core
everything is feature-major (features on SBUF partitions, batch on the free
axis), so each matmul is W_tile.T @ X^T with stationary weights. ALL matmuls
run in fp8e4 with perf_mode=DoubleRow (2 k-tiles per matmul, 2x effective
tensor throughput at FD=512): weights are scaled by 64 host-side to clear
the e4m3 denormal floor (x and activations are O(1), left unscaled), so the
whole pre-activation pipeline lives in a x64 domain; the single 1/64
compensation folds into each activation instruction's `scale` input, so the
instruction mix is unchanged. Activations are fp8 [128, k, CH] tiles; PSUM
accumulation is f32 throughout and evictions/bias adds run in f32/bf16.
Host-side prep: hidden neurons are permuted so the three activation groups
are contiguous, hr/or are folded into the weight matrices, weights are
packed so each loads as one large contiguous DMA, and hb/ob are applied as
per-partition bias APs (hb pre-scaled by 64 into the scaled domain).
"""

import os

import numpy as np
import ml_dtypes

import concourse.bass as bass
import concourse.tile as tile
from concourse import bacc, mybir
from concourse.bass_utils import run_bass_kernel_spmd

B, NI, NH, NO = 8192, 512, 2048, 512
N_STEPS = 4
N_CORES = 8
BL = B // N_CORES          # batch rows per core
CH = 512                   # batch chunk (one PSUM bank of fp32)
NCH = BL // CH             # 2 chunks per core
KI = NI // 128             # 4 k-tiles over inputs
KPI = KI // 2              # 2 DoubleRow k-pairs over inputs
KH = NH // 128             # 16 k/m-tiles over hidden
KP = KH // 2               # 8 DoubleRow k-pairs over hidden
KO = NO // 128             # 4 m-tiles over outputs
WSCALE = 64.0              # fp8 weight pre-scale (clears e4m3 denormal floor)
INV = 1.0 / WSCALE

BF16 = mybir.dt.bfloat16
F32 = mybir.dt.float32
FP8 = mybir.dt.float8e4
AF = mybir.ActivationFunctionType
DROW = mybir.MatmulPerfMode.DoubleRow

# hidden neurons regrouped as [all tanh | all sigmoid | all relu]
_idx = np.arange(NH)
PERM = np.concatenate([_idx[_idx % 3 == 0], _idx[_idx % 3 == 1], _idx[_idx % 3 == 2]])
_B1 = int((_idx % 3 == 0).sum())           # 683
_B2 = _B1 + int((_idx % 3 == 1).sum())     # 1366

# per m-tile: the single activation function, or None for the two mixed tiles
_TILE_FUNC = []
for _m in range(KH):
    _lo, _hi = _m * 128, (_m + 1) * 128
    _fs = set()
    for _f, _a, _b in ((AF.Tanh, 0, _B1), (AF.Sigmoid, _B1, _B2), (AF.Relu, _B2, NH)):
        if max(_lo, _a) < min(_hi, _b):
            _fs.add(_f)
    _TILE_FUNC.append(_fs.pop() if len(_fs) == 1 else None)

# mixed tiles: (major_func applied everywhere, minor_func, mask column block)
# partition sub-ranges must be 32-aligned on TRN2, so the minority strip is
# fixed up with a full-tile ACT + copy_predicated against a {0,1} mask
_BOUNDARY = {
    _B1 // 128: (AF.Sigmoid, AF.Tanh, 0),    # tile 5: parts < 43 are tanh
    _B2 // 128: (AF.Sigmoid, AF.Relu, 1),    # tile 10: parts >= 86 are relu
}


def _emit_hidden_act(nc, ps, blk, a_new, tmp_pool, bmask_t):
    """Evict a 4-bank PSUM block through the grouped activations into a_new.

    ps:    AP (128, 4*CH) f32/bf16 holding x64-domain pre-activations
           (bias included) for m-tiles blk*4..blk*4+3; the 1/64 rides in
           on the ACT scale input. Same-function runs fuse into wide ACTs.
    a_new: fp8 SBUF tile (128, KH, CH); m-tile m lives at [:, m, :]
    """
    mloc = 0
    while mloc < 4:
        m = blk * 4 + mloc
        if m in _BOUNDARY:
            major, minor, mb = _BOUNDARY[m]
            nc.scalar.activation(
                a_new[:, m:m + 1, :],
                ps[:, mloc * CH:(mloc + 1) * CH], major, scale=INV)
            t = tmp_pool.tile([128, CH], FP8, tag="btmp", bufs=2, name="btmp")
            nc.scalar.activation(t[:], ps[:, mloc * CH:(mloc + 1) * CH], minor,
                                 scale=INV)
            nc.vector.copy_predicated(
                a_new[:, m:m + 1, :],
                bmask_t[:, mb * CH:(mb + 1) * CH], t[:])
            mloc += 1
            continue
        func = _TILE_FUNC[m]
        end = mloc + 1
        while end < 4 and _TILE_FUNC[blk * 4 + end] == func:
            end += 1
        nc.scalar.activation(
            a_new[:, blk * 4 + mloc:blk * 4 + end, :],
            ps[:, mloc * CH:end * CH], func, scale=INV)
        mloc = end


def _build_nc():
    nc = bacc.Bacc("TRN2", target_bir_lowering=False, debug=False,
                   num_devices=N_CORES, dynamic_dma_scratch_size=2048)

    # all operands host-packed so each loads as one large contiguous DMA;
    # weight layouts are 3D [128, k, cols] so DoubleRow k-pairs slice as
    # [:, 2p:2p+2, :]. wih/whh row-block their k-pairs across dim 0.
    xT = nc.dram_tensor("xT", [128, KI * BL], FP8, kind="ExternalInput").ap()
    wih = nc.dram_tensor("wih", [2 * 128, 2, NH], FP8,
                         kind="ExternalInput").ap()
    whh = nc.dram_tensor("whh", [4 * 128, 4, NH], FP8,
                         kind="ExternalInput").ap()
    who = nc.dram_tensor("who", [128, KH, NO], FP8, kind="ExternalInput").ap()
    wio = nc.dram_tensor("wio", [128, KI, NO], FP8, kind="ExternalInput").ap()
    hbc = nc.dram_tensor("hbc", [128, KH], F32, kind="ExternalInput").ap()
    obc = nc.dram_tensor("obc", [128, KO], F32, kind="ExternalInput").ap()
    bmask = nc.dram_tensor("bmask", [128, 2 * CH], mybir.dt.uint8,
                           kind="ExternalInput").ap()
    outT = nc.dram_tensor("outT", [NO, BL], F32, kind="ExternalOutput").ap()

    with tile.TileContext(nc) as tc:
        with tc.tile_pool(name="w", bufs=1) as wpool, \
             tc.tile_pool(name="act", bufs=1) as apool, \
             tc.tile_pool(name="ps", bufs=2, space="PSUM") as pspool, \
             tc.tile_pool(name="out", bufs=4) as opool:

            # ---- stage inputs over 3 DMA queues (SP/ACT/POOL).
            # Critical path: wih+x(c0) unblock the PE at ~9us; the 4MB whh
            # (needed at ~27us) streams concurrently on 3 queues ----
            wih_m = []
            for p in range(KPI):
                t = wpool.tile([128, 2, NH], FP8, tag=f"wihP{p}",
                               name=f"wihP{p}")
                wih_m.append(t)
            x_m = wpool.tile([128, KI, BL], FP8, tag="x", name="xm")
            # first-block operands land first: both wih pairs' m0-3 columns
            # and all 4 chunk-0 x k-tiles, spread over all 3 queues
            for p in range(KPI):
                nc.sync.dma_start(wih_m[p][:, :, 0:4 * 128],
                                  wih[p * 128:(p + 1) * 128, :, 0:4 * 128])
            for k in range(2):
                nc.scalar.dma_start(x_m[:, k, 0:CH], xT[:, k * BL:k * BL + CH])
            for k in range(2, KI):
                nc.gpsimd.dma_start(x_m[:, k, 0:CH], xT[:, k * BL:k * BL + CH])
            for p in range(KPI):
                nc.sync.dma_start(wih_m[p][:, :, 4 * 128:NH],
                                  wih[p * 128:(p + 1) * 128, :, 4 * 128:NH])
            hbc_t = wpool.tile([128, KH], F32, tag="hbc")
            nc.gpsimd.dma_start(hbc_t[:], hbc[:])
            obc_t = wpool.tile([128, KO], F32, tag="obc")
            nc.gpsimd.dma_start(obc_t[:], obc[:])
            bmask_t = wpool.tile([128, 2 * CH], mybir.dt.uint8, tag="bmask")
            nc.gpsimd.dma_start(bmask_t[:], bmask[:])
            for k in range(KI):      # chunk-1 x, needed from ~16us
                eng = nc.scalar if k < 2 else nc.gpsimd
                eng.dma_start(x_m[:, k, CH:BL],
                              xT[:, k * BL + CH:(k + 1) * BL])
            wio_m = wpool.tile([128, KI, NO], FP8, tag="wio", name="wiom")
            nc.scalar.dma_start(wio_m[:], wio[:])
            whh_m = []
            for J in range(4):
                t = wpool.tile([128, 4, NH], FP8, tag=f"whhJ{J}",
                               name=f"whhJ{J}")
                eng = (nc.gpsimd, nc.gpsimd, nc.sync, nc.scalar)[J]
                eng.dma_start(t[:], whh[J * 128:(J + 1) * 128])
                whh_m.append(t)
            # DoubleRow k-pair kp covers k-tiles 2kp, 2kp+1 (same row block)
            whh_p2 = [whh_m[kp // 2][:, (kp % 2) * 2:(kp % 2) * 2 + 2, :]
                      for kp in range(KP)]

            # ---- per-chunk x-projection P64 = 64*(x@Wih.T + hb) and
            # first-step activations ----
            P = {}
            A = {}
            for c in range(NCH):
                P[c] = apool.tile([128, KH * CH], BF16, tag=f"P{c}",
                                  name=f"P{c}")
                a1 = apool.tile([128, KH, CH], FP8, tag="A8", bufs=3,
                                name=f"A1c{c}")
                for blk in range(4):
                    ps = pspool.tile([128, 4 * CH], F32, tag="ps", name="psb")
                    for p in range(KPI):
                        for mloc in range(4):
                            m = blk * 4 + mloc
                            nc.tensor.matmul(
                                ps[:, mloc * CH:(mloc + 1) * CH],
                                wih_m[p][:, :, m * 128:(m + 1) * 128],
                                x_m[:, 2 * p:2 * p + 2, c * CH:(c + 1) * CH],
                                start=(p == 0), stop=(p == KPI - 1),
                                perf_mode=DROW)
                    # P64 = psum + 64*hb in one wide DVE op (bias
                    # broadcast along the free axis); frees the PSUM slot
                    # as fast as a plain copy
                    hb4 = hbc_t[:, blk * 4:(blk + 1) * 4]
                    nc.vector.scalar_tensor_tensor(
                        P[c][:, blk * 4 * CH:(blk + 1) * 4 * CH], ps[:], 1.0,
                        hb4[:, :, None].broadcast_to([128, 4, CH]),
                        mybir.AluOpType.mult, mybir.AluOpType.add)
                    # A1 = act(P64/64) straight from SBUF
                    _emit_hidden_act(nc, P[c][:, blk * 4 * CH:(blk + 1) * 4 * CH],
                                     blk, a1, opool, bmask_t)
                A[c] = a1

            # ---- whh-independent output x-projection (fills the window
            # while the whh load is still in flight) ----
            outx = {}
            for c in range(NCH):
                outx[c] = apool.tile([128, KO * CH], BF16, tag=f"outx{c}",
                                     name=f"outx{c}")
                ps = pspool.tile([128, 4 * CH], F32, tag="ps", name="psb")
                for p in range(KPI):
                    for mo in range(KO):
                        nc.tensor.matmul(
                            ps[:, mo * CH:(mo + 1) * CH],
                            wio_m[:, 2 * p:2 * p + 2, mo * 128:(mo + 1) * 128],
                            x_m[:, 2 * p:2 * p + 2, c * CH:(c + 1) * CH],
                            start=(p == 0), stop=(p == KPI - 1),
                            perf_mode=DROW)
                nc.vector.tensor_copy(outx[c][:], ps[:])

            # ---- recurrent steps 2..4: fp8 DoubleRow over k-pairs ----
            def hh_step(c, s):
                a_new = apool.tile([128, KH, CH], FP8, tag="A8", bufs=3,
                                   name=f"A{s + 2}c{c}")
                for blk in range(4):
                    ps = pspool.tile([128, 4 * CH], F32, tag="ps", name="psb")
                    for kp in range(KP):
                        for mloc in range(4):
                            m = blk * 4 + mloc
                            nc.tensor.matmul(
                                ps[:, mloc * CH:(mloc + 1) * CH],
                                whh_p2[kp][:, :, m * 128:(m + 1) * 128],
                                A[c][:, 2 * kp:2 * kp + 2, :],
                                start=(kp == 0), stop=(kp == KP - 1),
                                perf_mode=DROW)
                    # pre64 = psum + P64 into an SBUF temp: a single PSUM
                    # read frees the bank; ACT then runs off SBUF
                    tmp = opool.tile([128, 4 * CH], F32, tag="pre", bufs=2,
                                     name="pre")
                    nc.vector.tensor_add(
                        tmp[:], ps[:], P[c][:, blk * 4 * CH:(blk + 1) * 4 * CH])
                    _emit_hidden_act(nc, tmp, blk, a_new, opool, bmask_t)
                A[c] = a_new

            for s in range(N_STEPS - 2):
                for c in range(NCH):
                    hh_step(c, s)
            hh_step(0, N_STEPS - 2)  # chunk 1's final step emitted after who

            # ---- output layer; chunk 0's output overlaps chunk 1's final
            # hh step ----
            who_m = wpool.tile([128, KH, NO], FP8, tag="who", name="whom")
            nc.scalar.dma_start(who_m[:], who[:])

            def out_chunk(c):
                for mo in range(KO):
                    pso = pspool.tile([128, CH], F32, tag="ps", name="pso")
                    oap = pso[:]
                    for kp in range(KP):
                        nc.tensor.matmul(
                            oap,
                            who_m[:, 2 * kp:2 * kp + 2,
                                  mo * 128:(mo + 1) * 128],
                            A[c][:, 2 * kp:2 * kp + 2, :],
                            start=(kp == 0), stop=(kp == KP - 1),
                            perf_mode=DROW)
                    to = opool.tile([128, CH], F32, tag="preo", bufs=2,
                                    name="preo")
                    nc.vector.tensor_add(
                        to[:], oap, outx[c][:, mo * CH:(mo + 1) * CH])
                    o = opool.tile([128, CH], F32, tag="o", bufs=2, name="o")
                    nc.scalar.activation(o[:], to[:], AF.Sigmoid,
                                         bias=obc_t[:, mo:mo + 1], scale=INV)
                    lo = c * CH
                    nc.sync.dma_start(
                        outT[mo * 128:(mo + 1) * 128, lo:lo + CH // 2],
                        o[:, 0:CH // 2])
                    nc.scalar.dma_start(
                        outT[mo * 128:(mo + 1) * 128, lo + CH // 2:lo + CH],
                        o[:, CH // 2:CH])

            hh_step(1, N_STEPS - 2)
            out_chunk(0)
            out_chunk(1)

    nc.compile()
    return nc


_NC_CACHE = None


def _get_nc():
    global _NC_CACHE
    if _NC_CACHE is None:
        _NC_CACHE = _build_nc()
    return _NC_CACHE


def _make_bmask():
    m = np.zeros((128, 2 * CH), np.uint8)
    m[:_B1 - (_B1 // 128) * 128, 0:CH] = 1          # tile 5: parts < 43 tanh
    m[_B2 - (_B2 // 128) * 128:, CH:2 * CH] = 1     # tile 10: parts >= 86 relu
    return m


def _q8(w):
    """Scale into e4m3 normal range and quantize (values stay << 240)."""
    return np.clip(w * WSCALE, -240.0, 240.0).astype(ml_dtypes.float8_e4m3)


def _prep_in_maps(inputs):
    f8 = ml_dtypes.float8_e4m3
    x = np.asarray(inputs["inputs"], np.float32)
    hr = np.asarray(inputs["hidden_responses"], np.float32)[PERM]
    hb = np.asarray(inputs["hidden_biases"], np.float32)[PERM]
    orr = np.asarray(inputs["output_responses"], np.float32)
    ob = np.asarray(inputs["output_biases"], np.float32)

    wih_s = (hr[:, None] * np.asarray(inputs["input_to_hidden"], np.float32)[PERM]).T
    whh_s = (hr[:, None] *
             np.asarray(inputs["hidden_to_hidden"], np.float32)[PERM][:, PERM]).T
    who_s = (orr[:, None] *
             np.asarray(inputs["hidden_to_output"], np.float32)[:, PERM]).T
    wio_s = (orr[:, None] * np.asarray(inputs["input_to_output"], np.float32)).T

    def pack3(w, ktiles):     # (ktiles*128, C) -> (128, ktiles, C)
        c = w.shape[1]
        return np.ascontiguousarray(w.reshape(ktiles, 128, c).transpose(1, 0, 2))

    # wih: row-block p packs k-tiles 2p, 2p+1; whh: row-block J packs 4J..4J+3
    wih_p = wih_s.reshape(2, 2, 128, NH).transpose(0, 2, 1, 3).reshape(2 * 128, 2, NH)
    whh_p = whh_s.reshape(4, 4, 128, NH).transpose(0, 2, 1, 3).reshape(4 * 128, 4, NH)

    shared = {
        "wih": _q8(wih_p),
        "whh": _q8(whh_p),
        "who": _q8(pack3(who_s, KH)),
        "wio": _q8(pack3(wio_s, KI)),
        "hbc": np.ascontiguousarray(hb.reshape(KH, 128).T) * np.float32(WSCALE),
        "obc": np.ascontiguousarray(ob.reshape(KO, 128).T),
        "bmask": _make_bmask(),
    }
    in_maps = []
    for c in range(N_CORES):
        m = dict(shared)
        xtc = np.ascontiguousarray(x[c * BL:(c + 1) * BL].T)  # (NI, BL)
        m["xT"] = np.ascontiguousarray(
            xtc.reshape(KI, 128, BL).transpose(1, 0, 2).reshape(128, KI * BL)
        ).astype(f8)
        in_maps.append(m)
    return in_maps


def _run(inputs, trace=False, tmpdir=None):
    import time

    nc = _get_nc()
    in_maps = _prep_in_maps(inputs)
    res = None
    for attempt in range(3):
        try:
            res = run_bass_kernel_spmd(nc, in_maps,
                                       core_ids=list(range(N_CORES)),
                                       trace=trace, tmpdir=tmpdir)
            break
        except Exception:
            # the device occasionally reports a transient unrecoverable
            # state on the first touch after another process released it
            if attempt == 2:
                raise
            try:
                import jax
                jax.clear_caches()
            except Exception:
                pass
            time.sleep(10)
    out = np.empty((B, NO), np.float32)
    for c in range(N_CORES):
        out[c * BL:(c + 1) * BL] = res.results[c]["outT"].T
    return out, res


def kernel(**inputs) -> np.ndarray:
    out, _ = _run(inputs, trace=False)
    return out


if __name__ == "__main__":
    rng = np.random.default_rng(0)
    ins = {
        "inputs": rng.standard_normal((B, NI), dtype=np.float32),
        "input_to_hidden": rng.standard_normal((NH, NI), dtype=np.float32) * 0.02,
        "hidden_to_hidden": rng.standard_normal((NH, NH), dtype=np.float32) * 0.02,
        "output_to_hidden": rng.standard_normal((NH, NO), dtype=np.float32) * 0.02,
        "input_to_output": rng.standard_normal((NO, NI), dtype=np.float32) * 0.02,
        "hidden_to_output": rng.standard_normal((NO, NH), dtype=np.float32) * 0.02,
        "output_to_output": rng.standard_normal((NO, NO), dtype=np.float32) * 0.02,
        "hidden_responses": rng.standard_normal(NH, dtype=np.float32) * 0.1 + 1.0,
        "hidden_biases": rng.standard_normal(NH, dtype=np.float32) * 0.1,
        "output_responses": rng.standard_normal(NO, dtype=np.float32) * 0.1 + 1.0,
        "output_biases": rng.standard_normal(NO, dtype=np.float32) * 0.1,
    }
    out = kernel(**ins)
    print("kernel output", out.shape, out.dtype, out[:2, :4])
